# revision 23
# baseline (speedup 1.0000x reference)
"""GATv2 (3-layer, heads=4/4/1) full-graph kernel for 8 Trainium2 NeuronCores.

Contract: kernel(**inputs) takes the FULL unsharded inputs (as produced by
setup_inputs()) and returns the FULL [64, 64] float32 output.

Structure (v2):
- Nodes padded to 50176 = 392 tiles of 128; 49 tiles per core. Edges assigned
  to the core owning their target node, sorted by target.
- xl (source transform) is stored per layer in two tile-range chunks
  (tiles 0-24 / 25-48 of each core) so the AllGather of each chunk can launch
  as soon as that tile range is done, overlapping with the rest of the edge
  phase. Row layout within a chunk is lane-major so gather rows for int16
  indices stay < 32768.
- Layer 0: xl0/xr0 computed replicated from host-pretransposed x0T (no AG).
- Layers 1,2: per-tile, right after the normalize of layer l-1, the node
  features are PE-transposed into an SBUF staging buffer and the next layer's
  Wl/Wr matmul runs immediately (own nodes only); chunk A is written+AllGathered
  at tile 24, chunk B + xr at tile 48. xn never touches DRAM node-major.
- Edge phase: dma_gather of xl[src] (one 2560-idx call per stream) and
  xr[tgt]; scores = att . leaky_relu(xl[src]+xr[tgt]) via DVE add + ACT Prelu
  + DVE mult/grouped-reduce; softmax without max-shift; scatter-sum and
  denominators via one-hot matmul on the PE into PSUM; per-node normalize;
  final global-mean-pool partials via PE, summed and divided on the host.
"""
import os
import numpy as np
import ml_dtypes

import concourse.bacc as bacc
import concourse.mybir as mybir
import concourse.tile as tile
from concourse._compat import get_trn_type
from concourse.bass_utils import run_bass_kernel_spmd

f16 = mybir.dt.float16
f32 = mybir.dt.float32
i16 = mybir.dt.int16
bf = ml_dtypes.bfloat16  # noqa: F401

P = 128
N = 50000
E = 800000
NP_ = 50176            # padded nodes = 392 * 128
NT = NP_ // P          # 392 global tiles
CORES = 8
NTC = NT // CORES      # 49 tiles per core
NC_NODES = NTC * P     # 6272 nodes per core
TA = 24                # tiles in chunk A
TB = NTC - TA          # 24 tiles in chunk B
RA = TA * P            # rows per core in chunk A
RB = TB * P
G_GRAPHS = 64
NEG = 0.2
GROUP = 2              # tiles per gather/DVE group

_CACHE = {}


def _pack_idx_image(seq):
    """int16 index sequence -> dma_gather SBUF image [128, len/16]."""
    n = len(seq)
    assert n % 128 == 0
    img = seq.reshape(n // 16, 16).T.astype(np.int16)  # [16, n/16]
    return np.tile(img, (8, 1))                        # [128, n/16]


def _preprocess(x, edge_index, batch, params):
    """Host-side: sort/pad edges, build all per-core tables and constants."""
    loops = np.arange(N, dtype=np.int64)
    src = np.concatenate([edge_index[0].astype(np.int64), loops])
    tgt = np.concatenate([edge_index[1].astype(np.int64), loops])
    order = np.argsort(tgt, kind="stable")
    srcs, tgts = src[order], tgt[order]

    # xl chunk-row of each source node (chunk by owning-core tile range)
    s_core = srcs // NC_NODES
    s_tt = (srcs % NC_NODES) // P
    s_lane = srcs % P
    isA = s_tt < TA
    rowA = s_core * RA + s_lane * TA + s_tt            # valid where isA
    rowB = s_core * RB + s_lane * TB + (s_tt - TA)     # valid where ~isA

    bounds = np.searchsorted(tgts, np.arange(0, NP_ + 1, P))
    nA = np.empty(NT, np.int64)
    nB = np.empty(NT, np.int64)
    for t in range(NT):
        s, e = bounds[t], bounds[t + 1]
        nA[t] = int(isA[s:e].sum())
        nB[t] = (e - s) - nA[t]
    CHA = int(max(1, -(-nA.max() // P)))   # ceil/128
    CHB = int(max(1, -(-nB.max() // P)))

    x_pad = np.zeros((NP_, x.shape[1]), np.float16)
    x_pad[:N] = x.astype(np.float16)
    x0T = np.ascontiguousarray(x_pad.T)                  # [128, NP_]

    iota_rep = np.tile(np.arange(P, dtype=np.float16)[None, :], (P, 1))
    ident = np.eye(P, dtype=np.float16)

    attds, wlrs = [], []
    for (Wl, Wr, att) in params:
        hc = Wl.shape[1]
        Wl_p = np.zeros((P, P), np.float16)
        Wr_p = np.zeros((P, P), np.float16)
        Wl_p[:, :hc] = Wl.astype(np.float16)
        Wr_p[:, :hc] = Wr.astype(np.float16)
        wlrs.append(np.concatenate([Wl_p, Wr_p], axis=1))  # [128, 256]
        h, cph = att.shape
        ad = np.zeros((P, 4), np.float16)
        for hh in range(h):
            ad[hh * cph:(hh + 1) * cph, hh] = att[hh].astype(np.float16)
        attds.append(ad)                                   # [128, 4]

    in_maps = []
    for c in range(CORES):
        t0 = c * NTC
        base = t0 * P
        xA = np.zeros((NTC, CHA * P), np.int64)
        xB = np.zeros((NTC, CHB * P), np.int64)
        xr_A = np.zeros((NTC, CHA * P), np.int64)
        xr_B = np.zeros((NTC, CHB * P), np.int64)
        tl_A = np.full((NTC, CHA * P), P, np.float16)
        tl_B = np.full((NTC, CHB * P), P, np.float16)
        for tt in range(NTC):
            t = t0 + tt
            s, e = bounds[t], bounds[t + 1]
            sl = tgts[s:e]
            a_m = isA[s:e]
            tloc_own = sl - base
            # xr_own row layout: lane-major perm within the core's 49 tiles
            xr_p = (tloc_own % P) * NTC + tloc_own // P
            k = int(a_m.sum()); k2 = (e - s) - k
            xA[tt, :k] = rowA[s:e][a_m]
            xr_A[tt, :k] = xr_p[a_m]
            tl_A[tt, :k] = (sl[a_m] - t * P).astype(np.float16)
            xB[tt, :k2] = rowB[s:e][~a_m]
            xr_B[tt, :k2] = xr_p[~a_m]
            tl_B[tt, :k2] = (sl[~a_m] - t * P).astype(np.float16)

        A_imgs, B_imgs, xr_imgs, tl_cols = [], [], [], []
        i = 0
        while i < NTC:
            g = min(GROUP, NTC - i)
            A_imgs.append(_pack_idx_image(xA[i:i + g].reshape(-1)))
            B_imgs.append(_pack_idx_image(xB[i:i + g].reshape(-1)))
            xr_seq = np.concatenate(
                [xr_A[i:i + g].reshape(-1), xr_B[i:i + g].reshape(-1)])
            xr_imgs.append(_pack_idx_image(xr_seq))
            tl_seq = np.concatenate(
                [tl_A[i:i + g].reshape(-1), tl_B[i:i + g].reshape(-1)])
            tl_cols.append(tl_seq.reshape(g * (CHA + CHB), P).T)
            i += g
        tloc_mat = np.concatenate(tl_cols, axis=1)  # [128, NTC*CT]

        pool = np.zeros((P, NTC, G_GRAPHS), np.float16)
        for tt in range(NTC):
            gn = base + tt * P + np.arange(P)
            valid = gn < N
            pool[valid, tt, batch[gn[valid]]] = 1.0

        in_maps.append({
            "x0T": x0T,
            "x0ownT": np.ascontiguousarray(
                x0T[:, c * NC_NODES:(c + 1) * NC_NODES]),
            "xlidxA": np.concatenate(A_imgs, axis=1),
            "xlidxB": np.concatenate(B_imgs, axis=1),
            "xridx": np.concatenate(xr_imgs, axis=1),
            "tloc": tloc_mat.astype(np.float32),
            "iota": iota_rep,
            "ident": ident,
            "attd0": attds[0], "attd1": attds[1], "attd2": attds[2],
            "wlr0": wlrs[0], "wlr1": wlrs[1], "wlr2": wlrs[2],
            "pooloh": pool,
        })

    meta = dict(CHA=CHA, CHB=CHB)
    return meta, in_maps


def _build(meta):
    CHA, CHB = meta["CHA"], meta["CHB"]
    CHT = CHA + CHB
    nc = bacc.Bacc(
        get_trn_type() or "TRN2",
        target_bir_lowering=False,
        debug=False,
        num_devices=CORES,
        dynamic_dma_scratch_size=32768,
    )
    inp = {}
    for name, shape, dt in [
        ("x0T", [P, NP_], f16),
        ("x0ownT", [P, NC_NODES], f16),
        ("xlidxA", [P, NTC * CHA * 8], i16),
        ("xlidxB", [P, NTC * CHB * 8], i16),
        ("xridx", [P, NTC * CHT * 8], i16),
        ("tloc", [P, NTC * CHT], f32),
        ("iota", [P, P], f16),
        ("ident", [P, P], f16),
        ("attd0", [P, 4], f16), ("attd1", [P, 4], f16),
        ("attd2", [P, 4], f16),
        ("wlr0", [P, 256], f16), ("wlr1", [P, 256], f16),
        ("wlr2", [P, 256], f16),
        ("pooloh", [P, NTC, G_GRAPHS], f16),
    ]:
        inp[name] = nc.dram_tensor(name, shape, dt, kind="ExternalInput")

    pooled = nc.dram_tensor("pooled", [G_GRAPHS, G_GRAPHS], f32,
                            kind="ExternalOutput")

    # xl chunk tensors per layer. Layer 0 is written locally (replicated
    # compute); layers 1,2 are AllGathered from per-core xlo chunks.
    xgA = [nc.dram_tensor(f"xgA{l}", [CORES * RA, P], f16,
                          addr_space="Local" if l == 0 else "Shared")
           for l in range(3)]
    xgB = [nc.dram_tensor(f"xgB{l}", [CORES * RB, P], f16,
                          addr_space="Local" if l == 0 else "Shared")
           for l in range(3)]
    xloA = [None] + [nc.dram_tensor(f"xloA{l}", [RA, P], f16) for l in (1, 2)]
    xloB = [None] + [nc.dram_tensor(f"xloB{l}", [RB, P], f16) for l in (1, 2)]
    xr_own = [nc.dram_tensor(f"xr_own{l}", [NC_NODES, P], f16)
              for l in range(3)]

    H_l = [4, 4, 1]

    with tile.TileContext(nc) as tc:
        with (
            tc.tile_pool(name="const", bufs=1) as cpool,
            tc.tile_pool(name="stage", bufs=3) as spool,
            tc.tile_pool(name="own", bufs=1) as opool,
            tc.tile_pool(name="edge", bufs=2) as epool,
            tc.tile_pool(name="small", bufs=3) as smpool,
            tc.tile_pool(name="psS", bufs=2, space="PSUM") as psS,
            tc.tile_pool(name="psP", bufs=1, space="PSUM") as psP,
            tc.tile_pool(name="psO", bufs=1, space="PSUM") as psO,
            tc.tile_pool(name="psT8", bufs=2, space="PSUM") as psT8,
            tc.tile_pool(name="psC", bufs=2, space="PSUM") as psC,
        ):
            iota_t = cpool.tile([P, P], f16, tag="iota")
            nc.sync.dma_start(out=iota_t[:], in_=inp["iota"][:])
            ident_t = cpool.tile([P, P], f16, tag="ident")
            nc.sync.dma_start(out=ident_t[:], in_=inp["ident"][:])
            pool_t = cpool.tile([P, NTC, G_GRAPHS], f16, tag="pool")
            nc.sync.dma_start(out=pool_t[:], in_=inp["pooloh"][:])
            wlr_t, att_t = [], []
            for l in range(3):
                w = cpool.tile([P, 256], f16, tag=f"wlr{l}")
                nc.sync.dma_start(out=w[:], in_=inp[f"wlr{l}"][:])
                wlr_t.append(w)
                a = cpool.tile([P, 4], f16, tag=f"att{l}")
                nc.sync.dma_start(out=a[:], in_=inp[f"attd{l}"][:])
                att_t.append(a)

            pool_psum = psP.tile([G_GRAPHS, G_GRAPHS], f32, space="PSUM")

            # ---- phase A, layer 0: replicated xl0 for all books, xr0 own ----
            STRIP = 4
            for b in range(CORES):
                stg = spool.tile([P, NTC, P], f16, tag="stg", bufs=2)
                for r0 in range(0, NTC, 2 * STRIP):
                    rw = min(2 * STRIP, NTC - r0)
                    t0 = b * NTC + r0
                    xs = spool.tile([P, 2 * STRIP * P], f16, tag="xstrip",
                                    name="xs")[:, :rw * P]
                    nc.sync.dma_start(
                        out=xs[:], in_=inp["x0T"][:, t0 * P:(t0 + rw) * P])
                    for j0 in range(0, rw, STRIP):
                        w_ = min(STRIP, rw - j0)
                        ps = psS.tile([P, w_, P], f32, space="PSUM",
                                      tag="pss", name="ps")
                        for j in range(w_):
                            nc.tensor.matmul(
                                out=ps[:, j, :],
                                lhsT=xs[:, (j0 + j) * P:(j0 + j + 1) * P],
                                rhs=wlr_t[0][:, :P], start=True, stop=True)
                        nc.scalar.copy(
                            out=stg[:, r0 + j0:r0 + j0 + w_, :], in_=ps[:])
                nc.sync.dma_start(
                    out=xgA[0][b * RA:(b + 1) * RA, :].rearrange(
                        "(p t) f -> p t f", p=P),
                    in_=stg[:, 0:TA, :])
                nc.sync.dma_start(
                    out=xgB[0][b * RB:(b + 1) * RB, :].rearrange(
                        "(p t) f -> p t f", p=P),
                    in_=stg[:, TA:NTC, :])
            stg = spool.tile([P, NTC, P], f16, tag="stg", bufs=2)
            for r0 in range(0, NTC, 2 * STRIP):
                rw = min(2 * STRIP, NTC - r0)
                xs = spool.tile([P, 2 * STRIP * P], f16, tag="xstrip",
                                name="xs")[:, :rw * P]
                nc.sync.dma_start(
                    out=xs[:], in_=inp["x0ownT"][:, r0 * P:(r0 + rw) * P])
                for j0 in range(0, rw, STRIP):
                    w_ = min(STRIP, rw - j0)
                    ps = psS.tile([P, w_, P], f32, space="PSUM", tag="pss",
                                  name="ps")
                    for j in range(w_):
                        nc.tensor.matmul(
                            out=ps[:, j, :],
                            lhsT=xs[:, (j0 + j) * P:(j0 + j + 1) * P],
                            rhs=wlr_t[0][:, P:], start=True, stop=True)
                    nc.scalar.copy(out=stg[:, r0 + j0:r0 + j0 + w_, :],
                                   in_=ps[:])
            nc.sync.dma_start(
                out=xr_own[0][:].rearrange("(p t) f -> p t f", p=P),
                in_=stg[:])

            # ---- layers ----
            # Each layer runs in two passes: pass 0 processes every group's
            # chunk-A work (sources in tiles 0-24, AllGathered early), pass 1
            # the chunk-B work plus scatter-combine, normalize and the next
            # layer's transforms. This keeps the B-side AllGather off the
            # critical path: while it flies, pass 0 of the next layer runs.
            for l in range(3):
                H = H_l[l]
                CW = P // H

                acc = opool.tile([P, NTC, P + 4], f32, tag="acc")
                if l < 2:
                    xnT = opool.tile([P, NC_NODES], f16, tag="xnT")
                    stg2 = opool.tile([P, NTC, P], f16, tag="stg2")
                    stgXR = opool.tile([P, NTC, P], f16, tag="stgXR")

                for sblk in (0, 1):
                  gi = 0
                  while gi < NTC:
                    g = min(GROUP, NTC - gi)
                    CHS = [CHA, CHB][sblk]
                    n1 = g * CHS * P         # idx per xl stream
                    c1 = gi * CHS * 8
                    c2 = (gi * CHT + sblk * g * CHA) * 8

                    iX = smpool.tile([P, n1 // 16], i16, tag="iX")
                    nc.sync.dma_start(
                        out=iX[:],
                        in_=[inp["xlidxA"], inp["xlidxB"]][sblk]
                        [:, c1:c1 + n1 // 16])
                    iR = smpool.tile([P, n1 // 16], i16, tag="iR")
                    nc.sync.dma_start(
                        out=iR[:], in_=inp["xridx"][:, c2:c2 + n1 // 16])
                    tlc = smpool.tile([P, g * CHS], f32, tag="tlc")
                    c3 = gi * CHT + sblk * g * CHA
                    nc.sync.dma_start(
                        out=tlc[:], in_=inp["tloc"][:, c3:c3 + g * CHS])

                    def gathers(out_t, in_ap, idx_t, slots):
                        k = 0
                        while k < slots:
                            n = min(1024, slots - k)
                            nc.gpsimd.dma_gather(
                                out_ap=out_t[:, k // P:(k + n) // P, :],
                                in_ap=in_ap,
                                idxs_ap=idx_t[:, k // 16:(k + n) // 16],
                                num_idxs=n, num_idxs_reg=n, elem_size=P)
                            k += n

                    xlg = epool.tile([P, g * CHB, P], f16, tag="xlg", bufs=3,
                                     name="xlg")[:, :g * CHS, :]
                    gathers(xlg, [xgA[l], xgB[l]][sblk][:], iX, n1)
                    xrg = epool.tile([P, g * CHB, P], f16, tag="xrg",
                                     name="xrg")[:, :g * CHS, :]
                    gathers(xrg, xr_own[l][:], iR, n1)

                    nch = g * CHS
                    u = epool.tile([P, g * CHB, P], f16, tag="u", bufs=1,
                                   name="u")[:, :nch, :]
                    nc.vector.tensor_tensor(
                        out=u[:], in0=xlg[:], in1=xrg[:],
                        op=mybir.AluOpType.add)
                    psc = psC.tile([P, g * CHB, 4], f32, space="PSUM", tag="psc",
                                   name="psc")[:, :nch, :]
                    rhs = epool.tile([P, g * CHB, P + 4], f16, tag="rhs", bufs=3,
                                     name="rhs")[:, :nch, :]
                    # scores: per-chunk PE transpose of u, batched prelu from
                    # PSUM, per-chunk matmul against attd
                    for k0 in range(0, nch, 8):
                        bs = min(8, nch - k0)
                        uT8 = psT8.tile([P, 8, P], f16, space="PSUM",
                                        tag="uT8")
                        for k in range(bs):
                            nc.tensor.transpose(
                                uT8[:, k, :], u[:, k0 + k, :], ident_t[:])
                        LT8 = smpool.tile([P, 8, P], f16, tag="LT8")
                        nc.scalar.activation(
                            out=LT8[:, :bs, :], in_=uT8[:, :bs, :],
                            func=mybir.ActivationFunctionType.Prelu,
                            alpha=NEG)
                        for k in range(bs):
                            nc.tensor.matmul(
                                out=psc[:, k0 + k, :],
                                lhsT=LT8[:, k, :], rhs=att_t[l][:],
                                start=True, stop=True)
                    nc.scalar.activation(
                        out=rhs[:, :, P:P + 4], in_=psc[:],
                        func=mybir.ActivationFunctionType.Exp)
                    p_b = rhs[:, :, P:P + H].unsqueeze(3).broadcast_to(
                        [P, nch, H, CW])
                    nc.vector.tensor_tensor(
                        out=rhs[:, :, :P].rearrange(
                            "p c (h w) -> p c h w", h=H),
                        in0=xlg[:].rearrange("p c (h w) -> p c h w", h=H),
                        in1=p_b,
                        op=mybir.AluOpType.mult)
                    oh = epool.tile([P, g * CHB, P], f16, tag="oh", bufs=3,
                                    name="oh")[:, :nch, :]
                    for c in range(nch):
                        nc.vector.tensor_scalar(
                            out=oh[:, c, :], in0=iota_t[:],
                            scalar1=tlc[:, c:c + 1], scalar2=None,
                            op0=mybir.AluOpType.is_equal)

                    for tt in range(g):
                        t = gi + tt
                        ps = psS.tile([P, P + 4], f32, space="PSUM",
                                      tag="pss")
                        for cc in range(CHS):
                            nc.tensor.matmul(
                                out=ps[:],
                                lhsT=oh[:, tt * CHS + cc, :],
                                rhs=rhs[:, tt * CHS + cc, :],
                                start=(cc == 0), stop=(cc == CHS - 1))
                        if sblk == 0:
                            nc.scalar.copy(out=acc[:, t, :], in_=ps[:])
                            continue
                        comb = smpool.tile([P, P + 4], f32, tag="comb")
                        nc.vector.tensor_tensor(
                            out=comb[:], in0=acc[:, t, :], in1=ps[:],
                            op=mybir.AluOpType.add)
                        den = smpool.tile([P, H], f32, tag="den")
                        nc.vector.tensor_scalar_max(
                            out=den[:], in0=comb[:, P:P + H], scalar1=1e-30)
                        rec = smpool.tile([P, H], f32, tag="rec")
                        nc.vector.reciprocal(out=rec[:], in_=den[:])
                        if l < 2:
                            t1 = smpool.tile([P, P], f16, tag="t1")
                            nc.vector.tensor_tensor(
                                out=t1[:].rearrange("p (h w) -> p h w", h=H),
                                in0=comb[:, :P].rearrange(
                                    "p (h w) -> p h w", h=H),
                                in1=rec[:].unsqueeze(2).broadcast_to(
                                    [P, H, CW]),
                                op=mybir.AluOpType.mult)
                            t1p = smpool.tile([P, P], f16, tag="t1p")
                            nc.scalar.activation(
                                out=t1p[:], in_=t1[:],
                                func=mybir.ActivationFunctionType.Prelu,
                                alpha=NEG)
                            pso = psO.tile([P, 384], f32, space="PSUM",
                                           tag="pso")
                            nc.tensor.matmul(
                                out=pso[:, 0:P], lhsT=t1p[:], rhs=ident_t[:],
                                start=True, stop=True)
                            nc.scalar.copy(
                                out=xnT[:, t * P:(t + 1) * P],
                                in_=pso[:, 0:P])
                            nc.tensor.matmul(
                                out=pso[:, P:P + 256],
                                lhsT=xnT[:, t * P:(t + 1) * P],
                                rhs=wlr_t[l + 1][:], start=True, stop=True)
                            nc.scalar.copy(out=stg2[:, t, :],
                                           in_=pso[:, P:2 * P])
                            nc.scalar.copy(out=stgXR[:, t, :],
                                           in_=pso[:, 2 * P:3 * P])
                            if t == TA - 1:
                                nc.sync.dma_start(
                                    out=xloA[l + 1][:].rearrange(
                                        "(p t) f -> p t f", p=P),
                                    in_=stg2[:, 0:TA, :])
                                nc.gpsimd.collective_compute(
                                    "AllGather", mybir.AluOpType.bypass,
                                    replica_groups=[list(range(CORES))],
                                    ins=[xloA[l + 1][:]],
                                    outs=[xgA[l + 1][:]])
                            if t == NTC - 1:
                                nc.sync.dma_start(
                                    out=xloB[l + 1][:].rearrange(
                                        "(p t) f -> p t f", p=P),
                                    in_=stg2[:, TA:NTC, :])
                                nc.gpsimd.collective_compute(
                                    "AllGather", mybir.AluOpType.bypass,
                                    replica_groups=[list(range(CORES))],
                                    ins=[xloB[l + 1][:]],
                                    outs=[xgB[l + 1][:]])
                                nc.sync.dma_start(
                                    out=xr_own[l + 1][:].rearrange(
                                        "(p t) f -> p t f", p=P),
                                    in_=stgXR[:])
                        else:
                            t1 = smpool.tile([P, H, CW], f32, tag="t1f")
                            nc.vector.tensor_tensor(
                                out=t1[:],
                                in0=comb[:, :P].rearrange(
                                    "p (h w) -> p h w", h=H),
                                in1=rec[:].unsqueeze(2).broadcast_to(
                                    [P, H, CW]),
                                op=mybir.AluOpType.mult)
                            xnm = smpool.tile([P, G_GRAPHS], f16, tag="xnm2")
                            nc.scalar.activation(
                                out=xnm[:],
                                in_=t1[:, 0, :G_GRAPHS],
                                func=mybir.ActivationFunctionType.Prelu,
                                alpha=NEG)
                            nc.tensor.matmul(
                                out=pool_psum[:],
                                lhsT=pool_t[:, t, :], rhs=xnm[:],
                                start=(t == 0), stop=(t == NTC - 1))
                    gi += g

            pool_sb = smpool.tile([G_GRAPHS, G_GRAPHS], f32, tag="poolsb")
            nc.vector.tensor_copy(out=pool_sb[:], in_=pool_psum[:])
            nc.sync.dma_start(out=pooled[:], in_=pool_sb[:])

    nc.finalize()
    return nc


def kernel(**inputs):
    x = np.asarray(inputs["x"])
    edge_index = np.asarray(inputs["edge_index"])
    batch = np.asarray(inputs["batch"])
    params = []
    for l in range(3):
        params.append((np.asarray(inputs[f"Wl{l}"]),
                       np.asarray(inputs[f"Wr{l}"]),
                       np.asarray(inputs[f"att{l}"])))
        b = np.asarray(inputs[f"b{l}"])
        assert np.all(b == 0), "nonzero bias not supported"

    meta, in_maps = _preprocess(x, edge_index, batch, params)

    key = ("nc", meta["CHA"], meta["CHB"])
    if key not in _CACHE:
        _CACHE[key] = _build(meta)
    nc = _CACHE[key]

    try:
        res = run_bass_kernel_spmd(
            nc, in_maps, core_ids=list(range(CORES)),
            trace=bool(os.environ.get("GAT_TRACE")))
    except ModuleNotFoundError:
        res = run_bass_kernel_spmd(nc, in_maps, core_ids=list(range(CORES)))
    kernel._last_result = res

    pooled = np.zeros((G_GRAPHS, G_GRAPHS), np.float64)
    for c in range(CORES):
        pooled += res.results[c]["pooled"].astype(np.float64)
    cnt = np.bincount(batch, minlength=G_GRAPHS).astype(np.float64)
    out = pooled / np.maximum(cnt, 1.0)[:, None]
    return out.astype(np.float32)


# revision 25
# speedup vs baseline: 1.0040x; 1.0040x over previous
"""GATv2 (3-layer, heads=4/4/1) full-graph kernel for 8 Trainium2 NeuronCores.

Contract: kernel(**inputs) takes the FULL unsharded inputs (as produced by
setup_inputs()) and returns the FULL [64, 64] float32 output.

Structure (v2):
- Nodes padded to 50176 = 392 tiles of 128; 49 tiles per core. Edges assigned
  to the core owning their target node, sorted by target.
- xl (source transform) is stored per layer in two tile-range chunks
  (tiles 0-24 / 25-48 of each core) so the AllGather of each chunk can launch
  as soon as that tile range is done, overlapping with the rest of the edge
  phase. Row layout within a chunk is lane-major so gather rows for int16
  indices stay < 32768.
- Layer 0: xl0/xr0 computed replicated from host-pretransposed x0T (no AG).
- Layers 1,2: per-tile, right after the normalize of layer l-1, the node
  features are PE-transposed into an SBUF staging buffer and the next layer's
  Wl/Wr matmul runs immediately (own nodes only); chunk A is written+AllGathered
  at tile 24, chunk B + xr at tile 48. xn never touches DRAM node-major.
- Edge phase: dma_gather of xl[src] (one 2560-idx call per stream) and
  xr[tgt]; scores = att . leaky_relu(xl[src]+xr[tgt]) via DVE add + ACT Prelu
  + DVE mult/grouped-reduce; softmax without max-shift; scatter-sum and
  denominators via one-hot matmul on the PE into PSUM; per-node normalize;
  final global-mean-pool partials via PE, summed and divided on the host.
"""
import os
import numpy as np
import ml_dtypes

import concourse.bacc as bacc
import concourse.mybir as mybir
import concourse.tile as tile
from concourse._compat import get_trn_type
from concourse.bass_utils import run_bass_kernel_spmd

f16 = mybir.dt.float16
f32 = mybir.dt.float32
i16 = mybir.dt.int16
bf = ml_dtypes.bfloat16  # noqa: F401

P = 128
N = 50000
E = 800000
NP_ = 50176            # padded nodes = 392 * 128
NT = NP_ // P          # 392 global tiles
CORES = 8
NTC = NT // CORES      # 49 tiles per core
NC_NODES = NTC * P     # 6272 nodes per core
TA = 22                # tiles in chunk A
TB = NTC - TA          # 24 tiles in chunk B
RA = TA * P            # rows per core in chunk A
RB = TB * P
G_GRAPHS = 64
NEG = 0.2
GROUP = 2              # tiles per gather/DVE group

_CACHE = {}


def _pack_idx_image(seq):
    """int16 index sequence -> dma_gather SBUF image [128, len/16]."""
    n = len(seq)
    assert n % 128 == 0
    img = seq.reshape(n // 16, 16).T.astype(np.int16)  # [16, n/16]
    return np.tile(img, (8, 1))                        # [128, n/16]


def _preprocess(x, edge_index, batch, params):
    """Host-side: sort/pad edges, build all per-core tables and constants."""
    loops = np.arange(N, dtype=np.int64)
    src = np.concatenate([edge_index[0].astype(np.int64), loops])
    tgt = np.concatenate([edge_index[1].astype(np.int64), loops])
    order = np.argsort(tgt, kind="stable")
    srcs, tgts = src[order], tgt[order]

    # xl chunk-row of each source node (chunk by owning-core tile range)
    s_core = srcs // NC_NODES
    s_tt = (srcs % NC_NODES) // P
    s_lane = srcs % P
    isA = s_tt < TA
    rowA = s_core * RA + s_lane * TA + s_tt            # valid where isA
    rowB = s_core * RB + s_lane * TB + (s_tt - TA)     # valid where ~isA

    bounds = np.searchsorted(tgts, np.arange(0, NP_ + 1, P))
    nA = np.empty(NT, np.int64)
    nB = np.empty(NT, np.int64)
    for t in range(NT):
        s, e = bounds[t], bounds[t + 1]
        nA[t] = int(isA[s:e].sum())
        nB[t] = (e - s) - nA[t]
    CHA = int(max(1, -(-nA.max() // P)))   # ceil/128
    CHB = int(max(1, -(-nB.max() // P)))

    x_pad = np.zeros((NP_, x.shape[1]), np.float16)
    x_pad[:N] = x.astype(np.float16)
    x0T = np.ascontiguousarray(x_pad.T)                  # [128, NP_]

    iota_rep = np.tile(np.arange(P, dtype=np.float16)[None, :], (P, 1))
    ident = np.eye(P, dtype=np.float16)

    attds, wlrs = [], []
    for (Wl, Wr, att) in params:
        hc = Wl.shape[1]
        Wl_p = np.zeros((P, P), np.float16)
        Wr_p = np.zeros((P, P), np.float16)
        Wl_p[:, :hc] = Wl.astype(np.float16)
        Wr_p[:, :hc] = Wr.astype(np.float16)
        wlrs.append(np.concatenate([Wl_p, Wr_p], axis=1))  # [128, 256]
        h, cph = att.shape
        ad = np.zeros((P, 4), np.float16)
        for hh in range(h):
            ad[hh * cph:(hh + 1) * cph, hh] = att[hh].astype(np.float16)
        attds.append(ad)                                   # [128, 4]

    in_maps = []
    for c in range(CORES):
        t0 = c * NTC
        base = t0 * P
        xA = np.zeros((NTC, CHA * P), np.int64)
        xB = np.zeros((NTC, CHB * P), np.int64)
        xr_A = np.zeros((NTC, CHA * P), np.int64)
        xr_B = np.zeros((NTC, CHB * P), np.int64)
        tl_A = np.full((NTC, CHA * P), P, np.float16)
        tl_B = np.full((NTC, CHB * P), P, np.float16)
        for tt in range(NTC):
            t = t0 + tt
            s, e = bounds[t], bounds[t + 1]
            sl = tgts[s:e]
            a_m = isA[s:e]
            tloc_own = sl - base
            # xr_own row layout: lane-major perm within the core's 49 tiles
            xr_p = (tloc_own % P) * NTC + tloc_own // P
            k = int(a_m.sum()); k2 = (e - s) - k
            xA[tt, :k] = rowA[s:e][a_m]
            xr_A[tt, :k] = xr_p[a_m]
            tl_A[tt, :k] = (sl[a_m] - t * P).astype(np.float16)
            xB[tt, :k2] = rowB[s:e][~a_m]
            xr_B[tt, :k2] = xr_p[~a_m]
            tl_B[tt, :k2] = (sl[~a_m] - t * P).astype(np.float16)

        A_imgs, B_imgs, xr_imgs, tl_cols = [], [], [], []
        i = 0
        while i < NTC:
            g = min(GROUP, NTC - i)
            A_imgs.append(_pack_idx_image(xA[i:i + g].reshape(-1)))
            B_imgs.append(_pack_idx_image(xB[i:i + g].reshape(-1)))
            xr_seq = np.concatenate(
                [xr_A[i:i + g].reshape(-1), xr_B[i:i + g].reshape(-1)])
            xr_imgs.append(_pack_idx_image(xr_seq))
            tl_seq = np.concatenate(
                [tl_A[i:i + g].reshape(-1), tl_B[i:i + g].reshape(-1)])
            tl_cols.append(tl_seq.reshape(g * (CHA + CHB), P).T)
            i += g
        tloc_mat = np.concatenate(tl_cols, axis=1)  # [128, NTC*CT]

        pool = np.zeros((P, NTC, G_GRAPHS), np.float16)
        for tt in range(NTC):
            gn = base + tt * P + np.arange(P)
            valid = gn < N
            pool[valid, tt, batch[gn[valid]]] = 1.0

        in_maps.append({
            "x0T": x0T,
            "x0ownT": np.ascontiguousarray(
                x0T[:, c * NC_NODES:(c + 1) * NC_NODES]),
            "xlidxA": np.concatenate(A_imgs, axis=1),
            "xlidxB": np.concatenate(B_imgs, axis=1),
            "xridx": np.concatenate(xr_imgs, axis=1),
            "tloc": tloc_mat.astype(np.float32),
            "iota": iota_rep,
            "ident": ident,
            "attd0": attds[0], "attd1": attds[1], "attd2": attds[2],
            "wlr0": wlrs[0], "wlr1": wlrs[1], "wlr2": wlrs[2],
            "pooloh": pool,
        })

    meta = dict(CHA=CHA, CHB=CHB)
    return meta, in_maps


def _build(meta):
    CHA, CHB = meta["CHA"], meta["CHB"]
    CHT = CHA + CHB
    nc = bacc.Bacc(
        get_trn_type() or "TRN2",
        target_bir_lowering=False,
        debug=False,
        num_devices=CORES,
        dynamic_dma_scratch_size=32768,
    )
    inp = {}
    for name, shape, dt in [
        ("x0T", [P, NP_], f16),
        ("x0ownT", [P, NC_NODES], f16),
        ("xlidxA", [P, NTC * CHA * 8], i16),
        ("xlidxB", [P, NTC * CHB * 8], i16),
        ("xridx", [P, NTC * CHT * 8], i16),
        ("tloc", [P, NTC * CHT], f32),
        ("iota", [P, P], f16),
        ("ident", [P, P], f16),
        ("attd0", [P, 4], f16), ("attd1", [P, 4], f16),
        ("attd2", [P, 4], f16),
        ("wlr0", [P, 256], f16), ("wlr1", [P, 256], f16),
        ("wlr2", [P, 256], f16),
        ("pooloh", [P, NTC, G_GRAPHS], f16),
    ]:
        inp[name] = nc.dram_tensor(name, shape, dt, kind="ExternalInput")

    pooled = nc.dram_tensor("pooled", [G_GRAPHS, G_GRAPHS], f32,
                            kind="ExternalOutput")

    # xl chunk tensors per layer. Layer 0 is written locally (replicated
    # compute); layers 1,2 are AllGathered from per-core xlo chunks.
    xgA = [nc.dram_tensor(f"xgA{l}", [CORES * RA, P], f16,
                          addr_space="Local" if l == 0 else "Shared")
           for l in range(3)]
    xgB = [nc.dram_tensor(f"xgB{l}", [CORES * RB, P], f16,
                          addr_space="Local" if l == 0 else "Shared")
           for l in range(3)]
    xloA = [None] + [nc.dram_tensor(f"xloA{l}", [RA, P], f16) for l in (1, 2)]
    xloB = [None] + [nc.dram_tensor(f"xloB{l}", [RB, P], f16) for l in (1, 2)]
    xr_own = [nc.dram_tensor(f"xr_own{l}", [NC_NODES, P], f16)
              for l in range(3)]

    H_l = [4, 4, 1]

    with tile.TileContext(nc) as tc:
        with (
            tc.tile_pool(name="const", bufs=1) as cpool,
            tc.tile_pool(name="stage", bufs=3) as spool,
            tc.tile_pool(name="own", bufs=1) as opool,
            tc.tile_pool(name="edge", bufs=2) as epool,
            tc.tile_pool(name="small", bufs=3) as smpool,
            tc.tile_pool(name="psS", bufs=2, space="PSUM") as psS,
            tc.tile_pool(name="psP", bufs=1, space="PSUM") as psP,
            tc.tile_pool(name="psO", bufs=1, space="PSUM") as psO,
            tc.tile_pool(name="psT8", bufs=2, space="PSUM") as psT8,
            tc.tile_pool(name="psC", bufs=2, space="PSUM") as psC,
        ):
            iota_t = cpool.tile([P, P], f16, tag="iota")
            nc.sync.dma_start(out=iota_t[:], in_=inp["iota"][:])
            ident_t = cpool.tile([P, P], f16, tag="ident")
            nc.sync.dma_start(out=ident_t[:], in_=inp["ident"][:])
            pool_t = cpool.tile([P, NTC, G_GRAPHS], f16, tag="pool")
            nc.sync.dma_start(out=pool_t[:], in_=inp["pooloh"][:])
            wlr_t, att_t = [], []
            for l in range(3):
                w = cpool.tile([P, 256], f16, tag=f"wlr{l}")
                nc.sync.dma_start(out=w[:], in_=inp[f"wlr{l}"][:])
                wlr_t.append(w)
                a = cpool.tile([P, 4], f16, tag=f"att{l}")
                nc.sync.dma_start(out=a[:], in_=inp[f"attd{l}"][:])
                att_t.append(a)

            pool_psum = psP.tile([G_GRAPHS, G_GRAPHS], f32, space="PSUM")

            # ---- phase A, layer 0: replicated xl0 for all books, xr0 own ----
            STRIP = 4
            for b in range(CORES):
                stg = spool.tile([P, NTC, P], f16, tag="stg", bufs=2)
                for r0 in range(0, NTC, 2 * STRIP):
                    rw = min(2 * STRIP, NTC - r0)
                    t0 = b * NTC + r0
                    xs = spool.tile([P, 2 * STRIP * P], f16, tag="xstrip",
                                    name="xs")[:, :rw * P]
                    nc.sync.dma_start(
                        out=xs[:], in_=inp["x0T"][:, t0 * P:(t0 + rw) * P])
                    for j0 in range(0, rw, STRIP):
                        w_ = min(STRIP, rw - j0)
                        ps = psS.tile([P, w_, P], f32, space="PSUM",
                                      tag="pss", name="ps")
                        for j in range(w_):
                            nc.tensor.matmul(
                                out=ps[:, j, :],
                                lhsT=xs[:, (j0 + j) * P:(j0 + j + 1) * P],
                                rhs=wlr_t[0][:, :P], start=True, stop=True)
                        nc.scalar.copy(
                            out=stg[:, r0 + j0:r0 + j0 + w_, :], in_=ps[:])
                nc.sync.dma_start(
                    out=xgA[0][b * RA:(b + 1) * RA, :].rearrange(
                        "(p t) f -> p t f", p=P),
                    in_=stg[:, 0:TA, :])
                nc.sync.dma_start(
                    out=xgB[0][b * RB:(b + 1) * RB, :].rearrange(
                        "(p t) f -> p t f", p=P),
                    in_=stg[:, TA:NTC, :])
            stg = spool.tile([P, NTC, P], f16, tag="stg", bufs=2)
            for r0 in range(0, NTC, 2 * STRIP):
                rw = min(2 * STRIP, NTC - r0)
                xs = spool.tile([P, 2 * STRIP * P], f16, tag="xstrip",
                                name="xs")[:, :rw * P]
                nc.sync.dma_start(
                    out=xs[:], in_=inp["x0ownT"][:, r0 * P:(r0 + rw) * P])
                for j0 in range(0, rw, STRIP):
                    w_ = min(STRIP, rw - j0)
                    ps = psS.tile([P, w_, P], f32, space="PSUM", tag="pss",
                                  name="ps")
                    for j in range(w_):
                        nc.tensor.matmul(
                            out=ps[:, j, :],
                            lhsT=xs[:, (j0 + j) * P:(j0 + j + 1) * P],
                            rhs=wlr_t[0][:, P:], start=True, stop=True)
                    nc.scalar.copy(out=stg[:, r0 + j0:r0 + j0 + w_, :],
                                   in_=ps[:])
            nc.sync.dma_start(
                out=xr_own[0][:].rearrange("(p t) f -> p t f", p=P),
                in_=stg[:])

            # ---- layers ----
            # Each layer runs in two passes: pass 0 processes every group's
            # chunk-A work (sources in tiles 0-24, AllGathered early), pass 1
            # the chunk-B work plus scatter-combine, normalize and the next
            # layer's transforms. This keeps the B-side AllGather off the
            # critical path: while it flies, pass 0 of the next layer runs.
            for l in range(3):
                H = H_l[l]
                CW = P // H

                acc = opool.tile([P, NTC, P + 4], f32, tag="acc")
                if l < 2:
                    xnT = opool.tile([P, NC_NODES], f16, tag="xnT")
                    stg2 = opool.tile([P, NTC, P], f16, tag="stg2")
                    stgXR = opool.tile([P, NTC, P], f16, tag="stgXR")

                for sblk in (0, 1):
                  gi = 0
                  while gi < NTC:
                    g = min(GROUP, NTC - gi)
                    CHS = [CHA, CHB][sblk]
                    n1 = g * CHS * P         # idx per xl stream
                    c1 = gi * CHS * 8
                    c2 = (gi * CHT + sblk * g * CHA) * 8

                    iX = smpool.tile([P, n1 // 16], i16, tag="iX")
                    nc.sync.dma_start(
                        out=iX[:],
                        in_=[inp["xlidxA"], inp["xlidxB"]][sblk]
                        [:, c1:c1 + n1 // 16])
                    iR = smpool.tile([P, n1 // 16], i16, tag="iR")
                    nc.sync.dma_start(
                        out=iR[:], in_=inp["xridx"][:, c2:c2 + n1 // 16])
                    tlc = smpool.tile([P, g * CHS], f32, tag="tlc")
                    c3 = gi * CHT + sblk * g * CHA
                    nc.sync.dma_start(
                        out=tlc[:], in_=inp["tloc"][:, c3:c3 + g * CHS])

                    def gathers(out_t, in_ap, idx_t, slots):
                        k = 0
                        while k < slots:
                            n = min(1024, slots - k)
                            nc.gpsimd.dma_gather(
                                out_ap=out_t[:, k // P:(k + n) // P, :],
                                in_ap=in_ap,
                                idxs_ap=idx_t[:, k // 16:(k + n) // 16],
                                num_idxs=n, num_idxs_reg=n, elem_size=P)
                            k += n

                    xlg = epool.tile([P, g * CHB, P], f16, tag="xlg", bufs=3,
                                     name="xlg")[:, :g * CHS, :]
                    gathers(xlg, [xgA[l], xgB[l]][sblk][:], iX, n1)
                    xrg = epool.tile([P, g * CHB, P], f16, tag="xrg",
                                     name="xrg")[:, :g * CHS, :]
                    gathers(xrg, xr_own[l][:], iR, n1)

                    nch = g * CHS
                    u = epool.tile([P, g * CHB, P], f16, tag="u", bufs=2,
                                   name="u")[:, :nch, :]
                    nc.vector.tensor_tensor(
                        out=u[:], in0=xlg[:], in1=xrg[:],
                        op=mybir.AluOpType.add)
                    psc = psC.tile([P, g * CHB, 4], f32, space="PSUM", tag="psc",
                                   name="psc")[:, :nch, :]
                    rhs = epool.tile([P, g * CHB, P + 4], f16, tag="rhs", bufs=3,
                                     name="rhs")[:, :nch, :]
                    # scores: per-chunk PE transpose of u, batched prelu from
                    # PSUM, per-chunk matmul against attd
                    for k0 in range(0, nch, 8):
                        bs = min(8, nch - k0)
                        uT8 = psT8.tile([P, 8, P], f16, space="PSUM",
                                        tag="uT8")
                        for k in range(bs):
                            nc.tensor.transpose(
                                uT8[:, k, :], u[:, k0 + k, :], ident_t[:])
                        LT8 = smpool.tile([P, 8, P], f16, tag="LT8")
                        nc.scalar.activation(
                            out=LT8[:, :bs, :], in_=uT8[:, :bs, :],
                            func=mybir.ActivationFunctionType.Prelu,
                            alpha=NEG)
                        for k in range(bs):
                            nc.tensor.matmul(
                                out=psc[:, k0 + k, :],
                                lhsT=LT8[:, k, :], rhs=att_t[l][:],
                                start=True, stop=True)
                    nc.scalar.activation(
                        out=rhs[:, :, P:P + 4], in_=psc[:],
                        func=mybir.ActivationFunctionType.Exp)
                    p_b = rhs[:, :, P:P + H].unsqueeze(3).broadcast_to(
                        [P, nch, H, CW])
                    nc.vector.tensor_tensor(
                        out=rhs[:, :, :P].rearrange(
                            "p c (h w) -> p c h w", h=H),
                        in0=xlg[:].rearrange("p c (h w) -> p c h w", h=H),
                        in1=p_b,
                        op=mybir.AluOpType.mult)
                    oh = epool.tile([P, g * CHB, P], f16, tag="oh", bufs=3,
                                    name="oh")[:, :nch, :]
                    for c in range(nch):
                        nc.vector.tensor_scalar(
                            out=oh[:, c, :], in0=iota_t[:],
                            scalar1=tlc[:, c:c + 1], scalar2=None,
                            op0=mybir.AluOpType.is_equal)

                    for tt in range(g):
                        t = gi + tt
                        ps = psS.tile([P, P + 4], f32, space="PSUM",
                                      tag="pss")
                        for cc in range(CHS):
                            nc.tensor.matmul(
                                out=ps[:],
                                lhsT=oh[:, tt * CHS + cc, :],
                                rhs=rhs[:, tt * CHS + cc, :],
                                start=(cc == 0), stop=(cc == CHS - 1))
                        if sblk == 0:
                            nc.scalar.copy(out=acc[:, t, :], in_=ps[:])
                            continue
                        comb = smpool.tile([P, P + 4], f32, tag="comb")
                        nc.vector.tensor_tensor(
                            out=comb[:], in0=acc[:, t, :], in1=ps[:],
                            op=mybir.AluOpType.add)
                        den = smpool.tile([P, H], f32, tag="den")
                        nc.vector.tensor_scalar_max(
                            out=den[:], in0=comb[:, P:P + H], scalar1=1e-30)
                        rec = smpool.tile([P, H], f32, tag="rec")
                        nc.vector.reciprocal(out=rec[:], in_=den[:])
                        if l < 2:
                            t1 = smpool.tile([P, P], f16, tag="t1")
                            nc.vector.tensor_tensor(
                                out=t1[:].rearrange("p (h w) -> p h w", h=H),
                                in0=comb[:, :P].rearrange(
                                    "p (h w) -> p h w", h=H),
                                in1=rec[:].unsqueeze(2).broadcast_to(
                                    [P, H, CW]),
                                op=mybir.AluOpType.mult)
                            t1p = smpool.tile([P, P], f16, tag="t1p")
                            nc.scalar.activation(
                                out=t1p[:], in_=t1[:],
                                func=mybir.ActivationFunctionType.Prelu,
                                alpha=NEG)
                            pso = psO.tile([P, 384], f32, space="PSUM",
                                           tag="pso")
                            nc.tensor.matmul(
                                out=pso[:, 0:P], lhsT=t1p[:], rhs=ident_t[:],
                                start=True, stop=True)
                            nc.scalar.copy(
                                out=xnT[:, t * P:(t + 1) * P],
                                in_=pso[:, 0:P])
                            nc.tensor.matmul(
                                out=pso[:, P:P + 256],
                                lhsT=xnT[:, t * P:(t + 1) * P],
                                rhs=wlr_t[l + 1][:], start=True, stop=True)
                            nc.scalar.copy(out=stg2[:, t, :],
                                           in_=pso[:, P:2 * P])
                            nc.scalar.copy(out=stgXR[:, t, :],
                                           in_=pso[:, 2 * P:3 * P])
                            if t == TA - 1:
                                nc.sync.dma_start(
                                    out=xloA[l + 1][:].rearrange(
                                        "(p t) f -> p t f", p=P),
                                    in_=stg2[:, 0:TA, :])
                                nc.gpsimd.collective_compute(
                                    "AllGather", mybir.AluOpType.bypass,
                                    replica_groups=[list(range(CORES))],
                                    ins=[xloA[l + 1][:]],
                                    outs=[xgA[l + 1][:]])
                            if t == NTC - 1:
                                nc.sync.dma_start(
                                    out=xloB[l + 1][:].rearrange(
                                        "(p t) f -> p t f", p=P),
                                    in_=stg2[:, TA:NTC, :])
                                nc.gpsimd.collective_compute(
                                    "AllGather", mybir.AluOpType.bypass,
                                    replica_groups=[list(range(CORES))],
                                    ins=[xloB[l + 1][:]],
                                    outs=[xgB[l + 1][:]])
                                nc.sync.dma_start(
                                    out=xr_own[l + 1][:].rearrange(
                                        "(p t) f -> p t f", p=P),
                                    in_=stgXR[:])
                        else:
                            t1 = smpool.tile([P, H, CW], f32, tag="t1f")
                            nc.vector.tensor_tensor(
                                out=t1[:],
                                in0=comb[:, :P].rearrange(
                                    "p (h w) -> p h w", h=H),
                                in1=rec[:].unsqueeze(2).broadcast_to(
                                    [P, H, CW]),
                                op=mybir.AluOpType.mult)
                            xnm = smpool.tile([P, G_GRAPHS], f16, tag="xnm2")
                            nc.scalar.activation(
                                out=xnm[:],
                                in_=t1[:, 0, :G_GRAPHS],
                                func=mybir.ActivationFunctionType.Prelu,
                                alpha=NEG)
                            nc.tensor.matmul(
                                out=pool_psum[:],
                                lhsT=pool_t[:, t, :], rhs=xnm[:],
                                start=(t == 0), stop=(t == NTC - 1))
                    gi += g

            pool_sb = smpool.tile([G_GRAPHS, G_GRAPHS], f32, tag="poolsb")
            nc.vector.tensor_copy(out=pool_sb[:], in_=pool_psum[:])
            nc.sync.dma_start(out=pooled[:], in_=pool_sb[:])

    nc.finalize()
    return nc


def kernel(**inputs):
    x = np.asarray(inputs["x"])
    edge_index = np.asarray(inputs["edge_index"])
    batch = np.asarray(inputs["batch"])
    params = []
    for l in range(3):
        params.append((np.asarray(inputs[f"Wl{l}"]),
                       np.asarray(inputs[f"Wr{l}"]),
                       np.asarray(inputs[f"att{l}"])))
        b = np.asarray(inputs[f"b{l}"])
        assert np.all(b == 0), "nonzero bias not supported"

    meta, in_maps = _preprocess(x, edge_index, batch, params)

    key = ("nc", meta["CHA"], meta["CHB"])
    if key not in _CACHE:
        _CACHE[key] = _build(meta)
    nc = _CACHE[key]

    try:
        res = run_bass_kernel_spmd(
            nc, in_maps, core_ids=list(range(CORES)),
            trace=bool(os.environ.get("GAT_TRACE")))
    except ModuleNotFoundError:
        res = run_bass_kernel_spmd(nc, in_maps, core_ids=list(range(CORES)))
    kernel._last_result = res

    pooled = np.zeros((G_GRAPHS, G_GRAPHS), np.float64)
    for c in range(CORES):
        pooled += res.results[c]["pooled"].astype(np.float64)
    cnt = np.bincount(batch, minlength=G_GRAPHS).astype(np.float64)
    out = pooled / np.maximum(cnt, 1.0)[:, None]
    return out.astype(np.float32)
